# revision 30
# baseline (speedup 1.0000x reference)
"""Multi-head attention (B=8, N=1024, DIM=1152, H=16, hd=72) on 8 TRN2 cores.

Sharding: pure data parallelism -- core i computes batch element i, weights
are replicated. No collectives.

Device-side strategy (per core):
  - x arrives bf16 (host cast); x^T is built by DMA-transpose during load
    (no PE/DVE cost).
  - Q^T, K^T computed in transposed layout [outdim, token] with bf16
    matmuls, stacked compactly in 128-row tiles (qkt).
  - Each head's 72-row Q^T/K^T slice is repacked to partition 0 by
    SBUF->SBUF DMAs (DMA shifts partitions freely; matmul operands must
    start at partition 0/32/64).
  - S^T = K_h @ Q_h^T puts softmax's k-reduction on PSUM partitions; the
    denominator is recovered free via a ones column appended to V
    (AV matmul emits [72+1, q], row 72 = sum_k exp).
  - exp on ScalarE over paired 1024-wide tiles with the 1/sqrt(hd) scale
    folded in; no max subtraction (scores are ~N(0,1), no overflow risk).
  - Normalization: denominator quadrant copied to SBUF, stream_shuffled to
    quadrant 0 (custom DVE ops require base partition 0),
    reciprocal_approx_fast, shuffled across quadrants, one fused DVE
    multiply into bf16 head-padded O^T.
  - Projection: per-head bf16 matmuls against host-zero-padded Wproj over
    the full 128 partitions (O^T pad rows zeroed once on GpSimd at t~0).
"""

import sys

sys.path.insert(0, "/opt/trn_rl_repo")

import numpy as np
import ml_dtypes

B, N, DIM, HEADS = 8, 1024, 1152, 16
HD = DIM // HEADS  # 72
NCORES = 8
QKDIM = 2 * DIM  # 2304 (q and k outdims concatenated)
N_MT_QK = QKDIM // 128  # 18 m-tiles for Q,K
N_KT = DIM // 128  # 9 contraction tiles
N_TT = N // 128  # 8 token tiles
QB = 512  # q block (moving dim) for S^T / qkv
N_QB = N // QB  # 2
VB = 288  # v block = 4 heads
N_VB = DIM // VB  # 4
EB = 384  # proj output block
N_EB = DIM // EB  # 3

_CACHE = {}


def _head_pieces(h):
    """Pieces covering rows [72h, 72h+72) of a 128-row-tiled stack, as
    (mtile, src_lo, src_hi, dst_lo): dest rows [dst_lo, dst_lo+src_hi-src_lo)
    come from src rows [src_lo, src_hi) of mtile."""
    r0 = HD * h
    mt, p0 = divmod(r0, 128)
    ln = min(HD, 128 - p0)
    pieces = [(mt, p0, p0 + ln, 0)]
    if ln < HD:
        pieces.append((mt + 1, 0, HD - ln, ln))
    return pieces


def _build(debug_taps=False):
    import concourse.tile as tile
    from concourse import bacc, mybir

    f32 = mybir.dt.float32
    bf16 = mybir.dt.bfloat16
    Exp = mybir.ActivationFunctionType.Exp

    nc = bacc.Bacc("TRN2", target_bir_lowering=False, debug=False,
                   num_devices=NCORES)

    x_d = nc.dram_tensor("x", [128, N_KT, N], bf16,
                         kind="ExternalInput").ap()  # x^T, host-relayouted
    wqk_d = nc.dram_tensor("wqk", [N_MT_QK, 128, N_KT, 128], bf16,
                           kind="ExternalInput").ap()
    wv_d = nc.dram_tensor("wv", [DIM, DIM], bf16, kind="ExternalInput").ap()
    bqk_d = nc.dram_tensor("bqk", [128, N_MT_QK], f32,
                           kind="ExternalInput").ap()
    bv_d = nc.dram_tensor("bv", [128, DIM], f32, kind="ExternalInput").ap()
    wproj_d = nc.dram_tensor("wproj", [128, HEADS, DIM], bf16,
                             kind="ExternalInput").ap()
    bproj_d = nc.dram_tensor("bproj", [128, DIM], f32,
                             kind="ExternalInput").ap()
    out_d = nc.dram_tensor("out", [N, DIM], f32, kind="ExternalOutput").ap()
    if debug_taps:
        qkt_d = nc.dram_tensor("dbg_qkt", [128, N_MT_QK, N], f32,
                               kind="ExternalOutput").ap()
        vpad_d = nc.dram_tensor("dbg_vpad", [128, N_TT, HEADS, HD + 1], f32,
                                kind="ExternalOutput").ap()
        ot_d = nc.dram_tensor("dbg_ot", [128, HEADS, N], f32,
                              kind="ExternalOutput").ap()
        xt_dbg_d = nc.dram_tensor("dbg_xt", [128, N_KT, N], f32,
                                  kind="ExternalOutput").ap()

    scale = float(HD) ** -0.5

    with tile.TileContext(nc) as tc:
        with tc.tile_pool(name="consts", bufs=1) as consts, \
             tc.tile_pool(name="persist", bufs=1) as persist:
            bqk_sb = consts.tile([128, N_MT_QK], f32)
            nc.sync.dma_start(bqk_sb, bqk_d)
            bv_sb = consts.tile([128, DIM], f32)
            nc.sync.dma_start(bv_sb, bv_d)
            bproj_sb = consts.tile([128, DIM], f32)
            nc.sync.dma_start(bproj_sb, bproj_d)

            warm = consts.tile([128, 128], bf16)
            nc.vector.memset(warm, 0.0)

            # Persistent activations
            qkt = persist.tile([128, N_MT_QK, N], bf16)      # Q^T,K^T stacked
            vpad = persist.tile([128, N_TT, HEADS, HD + 1], bf16)
            nc.vector.memset(vpad[:, :, :, HD:HD + 1], 1.0)  # denom trick

            # O^T lives for phases 2+3 but is allocated and zero-padded up
            # front so the memset runs on the idle GpSimd engine at t~0.
            ot_stack = tc.tile_pool(name="ot_pool", bufs=1)
            ot_pool = ot_stack.__enter__()
            ot = ot_pool.tile([128, HEADS, N], bf16)  # head-padded O^T
            nc.gpsimd.memset(ot[64:128, :, :], 0.0)

            # Repack staging opened at t0 so the head-0 repack DMAs can run
            # during the phase-1 tail.
            pad_stack = tc.tile_pool(name="qk_pad", bufs=1)
            qk_pad_pool = pad_stack.__enter__()

            # ---------------- Phase 1: x^T + QKV projections ---------------
            with tc.tile_pool(name="xt_pool", bufs=1) as xt_pool, \
                 tc.tile_pool(name="wv_pool", bufs=1) as wv_pool, \
                 tc.tile_pool(name="wqk_pool", bufs=4) as wqk_pool, \
                 tc.tile_pool(name="ph1_ps", bufs=1, space="PSUM") as ph1_ps:
                wps = ph1_ps.tile([128, QB], f32, tag="qk", bufs=4,
                                  name="wps")
                for _ in range(150):
                    nc.tensor.matmul(wps[:, 0:128], lhsT=warm, rhs=warm,
                                     start=True, stop=True)

                xt = xt_pool.tile([128, N_KT, N], bf16)  # x^T [dim, tok]
                # split by q-block so the first QK matmuls start after half
                nc.sync.dma_start(xt[:, :, 0:QB], x_d[:, :, 0:QB])
                nc.sync.dma_start(xt[:, :, QB:N], x_d[:, :, QB:N])
                wv_sb = wv_pool.tile([128, N_KT, DIM], bf16)

                # Q^T / K^T: m-tiles of 128 outdims
                for m in range(N_MT_QK):
                    if m == 4:
                        # wv load deferred behind the first wqk prefetches
                        nc.sync.dma_start(
                            wv_sb,
                            wv_d.rearrange("(kt kp) v -> kp kt v", kp=128))
                    w_t = wqk_pool.tile([128, N_KT, 128], bf16, tag="w")
                    nc.sync.dma_start(w_t, wqk_d[m])
                    for qb in range(N_QB):
                        ps = ph1_ps.tile([128, QB], f32, tag="qk", bufs=4)
                        for kt in range(N_KT):
                            nc.tensor.matmul(
                                ps,
                                lhsT=w_t[:, kt, :],
                                rhs=xt[:, kt, qb * QB:(qb + 1) * QB],
                                start=(kt == 0), stop=(kt == N_KT - 1))
                        nc.scalar.add(
                            qkt[:, m, qb * QB:(qb + 1) * QB], ps,
                            bqk_sb[:, m:m + 1])

                if debug_taps:
                    nc.gpsimd.dma_start(xt_dbg_d, xt)

                # V in natural layout, 4 heads (288 dims) per block;
                # vb outer so early heads' V completes first and AV can
                # overlap the V-phase tail
                for vb in range(N_VB):
                    for tt in range(N_TT):
                        ps = ph1_ps.tile([128, VB], f32, tag="v", bufs=4)
                        for kt in range(N_KT):
                            nc.tensor.matmul(
                                ps,
                                lhsT=xt[:, kt, tt * 128:(tt + 1) * 128],
                                rhs=wv_sb[:, kt, vb * VB:(vb + 1) * VB],
                                start=(kt == 0), stop=(kt == N_KT - 1))
                        nc.vector.tensor_add(
                            vpad[:, tt, 4 * vb:4 * vb + 4, 0:HD],
                            ps.rearrange("p (g d) -> p g d", g=4),
                            bv_sb[:, vb * VB:(vb + 1) * VB].rearrange(
                                "p (g d) -> p g d", g=4))

            if debug_taps:
                with tc.tile_pool(name="dbg_pool", bufs=2) as dbg_pool:
                    nc.gpsimd.dma_start(qkt_d, qkt)
                    for tt in range(N_TT):
                        cv = dbg_pool.tile([128, HEADS, HD + 1], f32, tag="cv")
                        nc.vector.tensor_copy(cv, vpad[:, tt])
                        nc.sync.dma_start(vpad_d[:, tt], cv)

            # Wproj is prefetched mid-attention (its space frees after
            # phase 1; issuing the DMAs at the boundary floods the queues
            # that the repack DMAs need).
            wp_stack = tc.tile_pool(name="wp_pool", bufs=1)
            wp_pool = wp_stack.__enter__()
            wp_sb = wp_pool.tile([128, HEADS, DIM], bf16)

            # ---------------- Phase 2: attention --------------------------
            # qb outer: after qb==0 all of O^T[:, :, 0:512] is final, so the
            # scheduler can interleave proj matmuls for token tiles 0..3
            # into the ACT-bound qb==1 window (proj pools are open below).
            with tc.tile_pool(name="es_pool", bufs=N_TT) as es_pool, \
                 tc.tile_pool(name="r_pool", bufs=3) as r_pool, \
                 tc.tile_pool(name="out_pool", bufs=3) as out_pool, \
                 tc.tile_pool(name="s_ps", bufs=2, space="PSUM") as s_ps, \
                 tc.tile_pool(name="o_ps", bufs=2, space="PSUM") as o_ps, \
                 tc.tile_pool(name="p_ps", bufs=2, space="PSUM") as p_ps:
                for qb in range(N_QB):
                    for h in range(HEADS):
                        pieces = _head_pieces(h)
                        # repack K^T rows of head h to partitions [0, 72)
                        # with SBUF->SBUF DMA (shifts partitions freely; DMA
                        # engines are idle during attention)
                        ktp = qk_pad_pool.tile([128, N], bf16, tag="ktp",
                                               bufs=3)
                        for (mt, lo, hi, dst) in pieces:
                            nc.sync.dma_start(
                                ktp[dst:dst + hi - lo, :],
                                qkt[lo:hi, 9 + mt, :])
                        if qb == 0 and h == 8:
                            # prefetch Wproj on the gpsimd queue mid-attention
                            for hh in range(HEADS):
                                nc.gpsimd.dma_start(
                                    wp_sb[:, hh, :], wproj_d[:, hh, :])
                        # repack Q^T rows of head h to partitions [0, 72)
                        qtp = qk_pad_pool.tile([128, QB], bf16, tag="qtp",
                                               bufs=4)
                        for (mt, lo, hi, dst) in pieces:
                            nc.gpsimd.dma_start(
                                qtp[dst:dst + hi - lo, :],
                                qkt[lo:hi, mt, qb * QB:(qb + 1) * QB])

                        es_tiles = []
                        for kp in range(N_TT // 2):
                            ps = s_ps.tile([128, 2, QB], f32, tag="s")
                            for j in range(2):
                                kt = 2 * kp + j
                                nc.tensor.matmul(
                                    ps[:, j, :],
                                    lhsT=ktp[0:HD, kt * 128:(kt + 1) * 128],
                                    rhs=qtp[0:HD, :],
                                    start=True, stop=True)
                            es = es_pool.tile([128, 2, QB], bf16, tag="e")
                            nc.scalar.activation(es, ps, func=Exp, scale=scale)
                            es_tiles.append(es)
                        ops = o_ps.tile([128, QB], f32, tag="o")
                        for kt in range(N_TT):
                            nc.tensor.matmul(
                                ops[0:HD + 1, :],
                                lhsT=vpad[:, kt, h, :],
                                rhs=es_tiles[kt // 2][:, kt % 2, :],
                                start=(kt == 0), stop=(kt == N_TT - 1))
                        # Broadcast 1/denominator (psum row 72) to rows 0..72
                        # at 32-aligned bases only: copy the quadrant to SBUF,
                        # shuffle lane 8 down to quadrant 0, invert there
                        # (custom DVE ops need base partition 0), then shuffle
                        # the reciprocal across quadrants.
                        rt = r_pool.tile([96, 2 * QB], f32, tag="r")
                        nc.vector.tensor_copy(
                            rt[64:96, 0:QB], ops[64:96, :])
                        nc.vector.stream_shuffle(
                            rt[0:32, 0:QB], rt[64:96, 0:QB], mask=[8] * 32)
                        nc.vector.reciprocal_approx_fast(
                            rt[0:32, QB:2 * QB], rt[0:32, 0:QB])
                        ident = list(range(32))
                        nc.vector.stream_shuffle(
                            rt[32:64, QB:2 * QB], rt[0:32, QB:2 * QB],
                            mask=ident)
                        nc.vector.stream_shuffle(
                            rt[64:96, QB:2 * QB], rt[0:32, QB:2 * QB],
                            mask=ident)
                        nc.vector.tensor_mul(
                            ot[0:HD, h, qb * QB:(qb + 1) * QB],
                            ops[0:HD, :], rt[0:HD, QB:2 * QB])

                # ------------- Phase 3: output projection -------------
                # Inside the same pool scope so proj matmuls for token tiles
                # 0..3 (ready after qb==0) can interleave with qb==1.
                for tt in range(N_TT):
                    outs = out_pool.tile([128, DIM], f32, tag="out")
                    for eb in range(N_EB):
                        ps = p_ps.tile([128, EB], f32, tag="p")
                        for h in range(HEADS):
                            nc.tensor.matmul(
                                ps,
                                lhsT=ot[:, h, tt * 128:(tt + 1) * 128],
                                rhs=wp_sb[:, h, eb * EB:(eb + 1) * EB],
                                start=(h == 0), stop=(h == HEADS - 1))
                        nc.vector.tensor_add(
                            outs[:, eb * EB:(eb + 1) * EB], ps,
                            bproj_sb[:, eb * EB:(eb + 1) * EB])
                    nc.sync.dma_start(out_d[tt * 128:(tt + 1) * 128, :], outs)

            if debug_taps:
                with tc.tile_pool(name="dbg_pool2", bufs=2) as dbg_pool2:
                    for hh in range(HEADS):
                        co = dbg_pool2.tile([128, N], f32, tag="co")
                        nc.vector.tensor_copy(co, ot[:, hh])
                        nc.sync.dma_start(ot_d[:, hh], co)

            wp_stack.__exit__(None, None, None)
            pad_stack.__exit__(None, None, None)
            ot_stack.__exit__(None, None, None)

    nc.compile()
    return nc


def _get_nc(debug_taps=False):
    key = ("nc", debug_taps)
    if key not in _CACHE:
        _CACHE[key] = _build(debug_taps)
    return _CACHE[key]


def _prep_shared(Wqkv, bqkv, Wproj, bproj):
    """Host-side pure-layout transforms of the (replicated) weights."""
    Wqkv = np.asarray(Wqkv, dtype=np.float32)
    bqkv = np.asarray(bqkv, dtype=np.float32)
    Wproj = np.asarray(Wproj, dtype=np.float32)
    bproj = np.asarray(bproj, dtype=np.float32)

    # [m, kp, kt, o]: per-m-tile, per-partition contiguous
    wqk = np.ascontiguousarray(
        Wqkv[:, :QKDIM].reshape(N_KT, 128, N_MT_QK, 128).transpose(2, 1, 0, 3)
    ).astype(ml_dtypes.bfloat16)
    wv = np.ascontiguousarray(Wqkv[:, QKDIM:]).astype(ml_dtypes.bfloat16)
    bqk = np.ascontiguousarray(bqkv[:QKDIM].reshape(N_MT_QK, 128).T)
    bv = np.ascontiguousarray(np.broadcast_to(bqkv[QKDIM:], (128, DIM)))
    # head-padded Wproj: rows [0:72, h] = Wproj[72h:72h+72], rest zero
    wproj = np.zeros((128, HEADS, DIM), dtype=ml_dtypes.bfloat16)
    for h in range(HEADS):
        wproj[0:HD, h, :] = Wproj[HD * h:HD * (h + 1), :].astype(
            ml_dtypes.bfloat16)
    bproj2 = np.ascontiguousarray(np.broadcast_to(bproj, (128, DIM)))
    return dict(wqk=wqk, wv=wv, bqk=bqk, bv=bv, wproj=wproj, bproj=bproj2)


def kernel(x, Wqkv, bqkv, Wproj, bproj, _trace=False, _debug_taps=False):
    from concourse import bass_utils

    x = np.asarray(x, dtype=np.float32).astype(ml_dtypes.bfloat16)
    # device layout: xt[p, dt, t] = x[t, dt*128+p]
    xt = x.transpose(0, 2, 1).reshape(NCORES, N_KT, 128, N).transpose(
        0, 2, 1, 3)
    shared = _prep_shared(Wqkv, bqkv, Wproj, bproj)
    in_maps = [dict(x=np.ascontiguousarray(xt[i]), **shared)
               for i in range(NCORES)]
    nc = _get_nc(_debug_taps)
    res = bass_utils.run_bass_kernel_spmd(
        nc, in_maps, core_ids=list(range(NCORES)), trace=_trace)
    out = np.stack([res.results[i]["out"] for i in range(NCORES)], axis=0)
    if _trace:
        _CACHE["last_exec_time_ns"] = res.exec_time_ns
        _CACHE["last_results"] = res
    return out
